# revision 6
# baseline (speedup 1.0000x reference)
"""Trainium2 Bass kernel v2 for nn_EquivariantRnn — batched time-parallel chains.

Key changes vs v1:
* 64 chains (8/core) instead of 8: each per-step matmul carries 8 columns,
  so the whole recurrence is dependency-latency bound, not instruction bound.
* fp16 matmul operands (1 cycle/row vs fp32's 4) with fp32 PSUM accumulation.
* Warmup shortened (contraction ~0.005/step measured) and most chains start
  from the host-computed mean-dynamics fixed point instead of zero.
* u = G0[seq] gathered host-side into the exact SBUF layout; one DMA stream.
* Final linears (launch 2) fp16 with single repacked weight DMAs; raw_emb
  (pure embedding-table sum) precomputed host-side.
"""

import os
import sys

for _p in ("/opt/trn_rl_repo", "/root/.axon_site/_ro/trn_rl_repo"):
    if _p not in sys.path and os.path.isdir(_p):
        sys.path.append(_p)

import numpy as np

import concourse.bass as bass
import concourse.tile as tile
import concourse.mybir as mybir
from concourse import bacc
from concourse.bass_utils import run_bass_kernel_spmd

B, T, IDX = 16, 64, 9
H, E = 512, 512
NCORES = 8
W = 8                    # chains per core
S = (IDX * B * T) // (NCORES * W)    # kept steps per chain (144)
L = 16                   # segment length (steps)
WU = 688                 # warmup steps  (WU + S divisible by L)
FP = mybir.dt.float32
F16 = mybir.dt.float16

NSTEPS = WU + S
assert NSTEPS % L == 0
NSEG = NSTEPS // L
NTOT = IDX * B * T

_cache = {}


def _run_with_retry(nc, in_maps, tries=3):
    import time as _time
    last = None
    for attempt in range(tries):
        try:
            return run_bass_kernel_spmd(nc, in_maps, core_ids=list(range(NCORES)))
        except Exception as e:  # noqa: BLE001
            last = e
            _time.sleep(10.0 * (attempt + 1))
    raise last


def build_launch1(nsteps=NSTEPS):
    """Recurrence launch. All tiles SBUF-resident. Layer 1 runs one step
    behind layer 0 and computes its Wih1 contribution per step (no bulk V),
    so there are no segment-boundary bursts on the PE. Each step's u-inject
    is hoisted into the previous step's latency gap."""
    nseg = nsteps // L
    C4 = 4 * W           # columns per step slot (32)

    nc = bacc.Bacc("TRN2", target_bir_lowering=False)
    # c1 pad-pattern boundaries: chain j of core 0 is padded while t < WU - j*S
    pat_bounds = sorted({WU - j * S for j in range(W)
                         if 0 < WU - j * S < nsteps})
    NPAT = len(pat_bounds) + 1
    # consts = ident(128) | h0i | h1i | c1pat(NPAT*C4) | useq head | wts(6144)
    # split so layer-0 can start before wt1/wtv arrive
    HEAD = min(2 * L, nsteps)
    XW = 128 + 2 * C4 + NPAT * C4 + HEAD * C4        # wts offset
    NCONST = XW + 6144
    const_d = nc.dram_tensor("consts", [128, NCONST], F16, kind="ExternalInput")
    useq_d = nc.dram_tensor("useq", [128, nsteps * C4], F16, kind="ExternalInput")
    h1out_d = nc.dram_tensor("h1out", [128, (nsteps - WU if nsteps > WU else nsteps) * C4],
                             F16, kind="ExternalOutput")
    kept0 = WU if nsteps > WU else 0     # first kept step

    with tile.TileContext(nc) as tc:
        with (tc.tile_pool(name="big", bufs=1) as big,
              tc.tile_pool(name="psp0", bufs=4, space="PSUM") as psp0,
              tc.tile_pool(name="psp1", bufs=4, space="PSUM") as psp1):
            consts = big.tile([128, NCONST], F16, name="consts")
            nc.sync.dma_start(consts[:, 0:XW + 2048], const_d.ap()[:, 0:XW + 2048])
            nc.sync.dma_start(consts[:, XW + 2048:], const_d.ap()[:, XW + 2048:])
            ident = consts[:, 0:128]
            c1pat = consts[:, 128 + 2 * C4:128 + 2 * C4 + NPAT * C4]
            uhead = consts[:, 128 + 2 * C4 + NPAT * C4:XW]
            wt0 = consts[:, XW:XW + 2048]
            wt1 = consts[:, XW + 2048:XW + 4096]
            wtv = consts[:, XW + 4096:XW + 6144]

            import bisect as _bisect

            def c1_at(t):
                idx = _bisect.bisect_right(pat_bounds, t)
                return c1pat[:, idx * C4:(idx + 1) * C4]

            useq = big.tile([128, nsteps * C4], F16, name="useq")
            # staged u DMAs (beyond the head packed into consts)
            bounds = [HEAD * C4]
            for frac in (6, 14, 30, nseg):
                b = min(frac, nseg) * L * C4
                if b > bounds[-1]:
                    bounds.append(b)
            for a, b in zip(bounds[:-1], bounds[1:]):
                nc.sync.dma_start(useq[:, a:b], useq_d.ap()[:, a:b])

            def u_at(t):
                if t < HEAD:
                    return uhead[:, t * C4:(t + 1) * C4]
                return useq[:, t * C4:(t + 1) * C4]

            h0reg = big.tile([128, (nsteps + 1) * C4], F16, name="h0reg")
            h1reg = big.tile([128, (nsteps + 1) * C4], F16, name="h1reg")
            nc.vector.tensor_copy(h0reg[:, 0:C4], consts[:, 128:128 + C4])
            nc.vector.tensor_copy(h1reg[:, 0:C4], consts[:, 128 + C4:128 + 2 * C4])
            h0r = h0reg[:].rearrange("p (t b w) -> p t b w", b=4, w=W)
            h1r = h1reg[:].rearrange("p (t b w) -> p t b w", b=4, w=W)

            ps0 = {}

            def inject0(t):
                # ready as soon as the psum bank is free: executes in the
                # latency gap before step t's accumulation
                ps = psp0.tile([128, C4], FP, tag="ps0", name="ps0")
                nc.tensor.matmul(ps[:, 0:C4], ident, u_at(t),
                                 start=True, stop=False)
                ps0[t] = ps

            def accum0(t):
                ps = ps0.pop(t)
                for i in range(4):
                    for j in range(4):
                        nc.tensor.matmul(
                            ps[:, i * W:(i + 1) * W],
                            wt0[:, (i * 4 + j) * 128:(i * 4 + j + 1) * 128],
                            h0r[:, t, j, :],
                            start=False, stop=(i == 3 and j == 3))
                nc.scalar.activation(h0reg[:, (t + 1) * C4:(t + 2) * C4],
                                     ps[:, 0:C4],
                                     mybir.ActivationFunctionType.Tanh,
                                     bias=0.0, scale=1.0)

            def emit_l1(t):
                # layer-1 step t: tanh(c1*mask + Wih1 @ h0[t+1] + Whh1 @ h1[t])
                ps = psp1.tile([128, C4], FP, tag="ps1", name="ps1")
                nc.tensor.matmul(ps[:, 0:C4], ident, c1_at(t),
                                 start=True, stop=False)
                for i in range(4):
                    for j in range(4):
                        nc.tensor.matmul(
                            ps[:, i * W:(i + 1) * W],
                            wtv[:, (i * 4 + j) * 128:(i * 4 + j + 1) * 128],
                            h0r[:, t + 1, j, :],
                            start=False, stop=False)
                for i in range(4):
                    for j in range(4):
                        nc.tensor.matmul(
                            ps[:, i * W:(i + 1) * W],
                            wt1[:, (i * 4 + j) * 128:(i * 4 + j + 1) * 128],
                            h1r[:, t, j, :],
                            start=False, stop=(i == 3 and j == 3))
                nc.scalar.activation(h1reg[:, (t + 1) * C4:(t + 2) * C4],
                                     ps[:, 0:C4],
                                     mybir.ActivationFunctionType.Tanh,
                                     bias=0.0, scale=1.0)

            # chunked output DMAs: issue as soon as each quarter of the kept
            # range is complete (layer-1 step t writes slot t+1)
            nkept = nsteps - kept0
            outpts = [kept0 + (nkept * q) // 4 for q in range(1, 4)]
            if nkept > 8:
                outpts.append(kept0 + nkept - 8)   # small final chunk → short tail

            def flush_out(upto_slot):
                # DMA kept slots (flush_out.done, upto_slot]
                a, b = flush_out.done, upto_slot
                if b > a:
                    nc.sync.dma_start(
                        h1out_d.ap()[:, (a - kept0) * C4:(b - kept0) * C4],
                        h1reg[:, (a + 1) * C4:(b + 1) * C4])
                    flush_out.done = b
            flush_out.done = kept0

            inject0(0)
            for t in range(nsteps):
                accum0(t)
                if t >= 1:
                    emit_l1(t - 1)
                    if t in outpts:       # L1 has completed slots up to t-1+1
                        flush_out(t)
                if t + 1 < nsteps:
                    inject0(t + 1)
            emit_l1(nsteps - 1)
            flush_out(nsteps)
    nc.compile()
    return nc


def build_launch2():
    """Final linears, token-parallel (128 tokens/core)."""
    nc = bacc.Bacc("TRN2", target_bir_lowering=False)
    # h1x = h1t(36*128) | raw(512) | bfin-row(512+128, partition 0 only)
    NX = 36 * 128 + 512 + 640
    h1x_d = nc.dram_tensor("h1x", [128, NX], F16, kind="ExternalInput")
    wfint_d = nc.dram_tensor("wfint", [128, 36 * 512], F16, kind="ExternalInput")
    out_d = nc.dram_tensor("out", [128, 512], FP, kind="ExternalOutput")

    with tile.TileContext(nc) as tc:
        with (tc.tile_pool(name="big", bufs=1) as big,
              tc.tile_pool(name="psf", bufs=1, space="PSUM") as psf):
            h1sb = big.tile([128, NX], F16, name="h1sb")
            nc.sync.dma_start(h1sb[:], h1x_d.ap())
            raw_sb = h1sb[:, 36 * 128:36 * 128 + 512]
            bfin_sb = h1sb[:, 36 * 128 + 512:]
            wfsb = big.tile([128, 36 * 512], F16, name="wfsb")
            # tapered chunks: small final chunk so the trailing matmuls
            # start as early as possible after the serial DMA stream
            wbounds = [0, 9, 18, 24, 30, 33, 36]
            for a, b in zip(wbounds[:-1], wbounds[1:]):
                nc.sync.dma_start(wfsb[:, a * 512:b * 512],
                                  wfint_d.ap()[:, a * 512:b * 512])

            pf = psf.tile([128, 512], FP, name="pf")
            nc.tensor.matmul(pf[:], bfin_sb[0:1, 512:512 + 128],
                             bfin_sb[0:1, 0:512], start=True, stop=False)
            for k in range(36):
                nc.tensor.matmul(pf[:], h1sb[:, k * 128:(k + 1) * 128],
                                 wfsb[:, k * 512:(k + 1) * 512],
                                 start=False, stop=(k == 35))

            gate = big.tile([128, 512], FP, name="gate")
            nc.vector.tensor_scalar(gate[:], pf[:], 0.0, 1.0,
                                    mybir.AluOpType.max, mybir.AluOpType.add)
            out_sb = big.tile([128, 512], FP, name="out_sb")
            nc.vector.tensor_mul(out_sb[:], gate[:], raw_sb[:])
            nc.sync.dma_start(out_d.ap(), out_sb[:])
    nc.compile()
    return nc


def _block_transpose_tiles(Wm):
    # [128, 16*128] f16: block (i, j) at col (i*4+j)*128 holds W[i*128+p, j*128+q] at [q, p]
    t = Wm.reshape(4, 128, 4, 128).transpose(0, 2, 3, 1).reshape(16, 128, 128)
    return np.ascontiguousarray(t.transpose(1, 0, 2).reshape(128, 16 * 128)).astype(np.float16)


def kernel(sequence, W_ad, b_ad, W_ly2, b_ly2, W_fin, b_fin,
           Wih0, Whh0, bih0, bhh0, Wih1, Whh1, bih1, bhh1, h_init):
    sequence = np.asarray(sequence)
    f32 = lambda x: np.asarray(x, dtype=np.float32)
    W_ad, b_ad, W_ly2, b_ly2 = f32(W_ad), f32(b_ad), f32(W_ly2), f32(b_ly2)
    W_fin, b_fin = f32(W_fin), f32(b_fin)
    Wih0, Whh0, bih0, bhh0 = f32(Wih0), f32(Whh0), f32(bih0), f32(bhh0)
    Wih1, Whh1, bih1, bhh1 = f32(Wih1), f32(Whh1), f32(bih1), f32(bhh1)
    h_init = f32(h_init)

    if "l1" not in _cache:
        _cache["l1"] = build_launch1()
    if "l2" not in _cache:
        _cache["l2"] = build_launch2()

    C4 = 4 * W
    G0 = ((W_ad.T @ Wih0.T) + (b_ad @ Wih0.T) + bih0 + bhh0).astype(np.float16)
    c1 = (bih1 + bhh1).astype(np.float32)

    wts = np.concatenate([_block_transpose_tiles(Whh0),
                          _block_transpose_tiles(Whh1),
                          _block_transpose_tiles(Wih1)], axis=1)
    identm = np.eye(128, dtype=np.float16)
    HEAD = min(2 * L, NSTEPS)
    pat_bounds = sorted({WU - j * S for j in range(W) if 0 < WU - j * S < NSTEPS})
    NPAT = len(pat_bounds) + 1
    c1bw = c1.reshape(4, 128).T.astype(np.float16)        # [128, 4]

    seq_flat = sequence.transpose(2, 0, 1).reshape(-1).astype(np.int64)
    assert seq_flat.shape[0] == NTOT

    in_maps = []
    for c in range(NCORES):
        g = c * W + np.arange(W)                       # global chain ids
        starts = g * S - WU                            # first (warmup) step
        tmat = starts[None, :] + np.arange(NSTEPS)[:, None]   # [nsteps, W]
        valid = tmat >= 0
        idx = np.where(valid, seq_flat[np.clip(tmat, 0, NTOT - 1)], 0)
        u = G0[idx] * valid[:, :, None].astype(np.float16)    # [nsteps, W, 512]
        useq = np.ascontiguousarray(
            u.reshape(NSTEPS, W, 4, 128).transpose(3, 0, 2, 1).reshape(128, NSTEPS * C4)
        ).astype(np.float16)
        # c1 pattern tiles: pattern idx applies for t in [bound[idx-1], bound[idx])
        # chain j active in pattern idx iff interval start >= (WU - g[j]*S)
        c1p = np.zeros((128, NPAT, 4, W), np.float16)
        for idx in range(NPAT):
            lo = 0 if idx == 0 else pat_bounds[idx - 1]
            act = (lo >= (WU - g * S)).astype(np.float16)     # [W]
            c1p[:, idx, :, :] = c1bw[:, :, None] * act[None, None, :]
        c1p = c1p.reshape(128, NPAT * C4)
        # zero init for warmup chains; chains that replay from global step 0
        # (starts <= 0) carry the true h_init instead
        full = starts > 0
        h0i = np.where(full[None, None, :], 0.0,
                       h_init[0].reshape(4, 128, 1).transpose(1, 0, 2)).astype(np.float32)
        h1i = np.where(full[None, None, :], 0.0,
                       h_init[1].reshape(4, 128, 1).transpose(1, 0, 2)).astype(np.float32)
        consts = np.concatenate([
            identm,
            h0i.reshape(128, C4).astype(np.float16),
            h1i.reshape(128, C4).astype(np.float16),
            c1p,
            useq[:, :HEAD * C4],
            wts,
        ], axis=1).astype(np.float16)
        in_maps.append({"consts": np.ascontiguousarray(consts), "useq": useq})

    res1 = _run_with_retry(_cache["l1"], in_maps)

    # reassemble: h1out [128, S, 4, W] -> h1_all [NTOT, H]
    h1_all = np.zeros((NTOT, H), np.float32)
    for c in range(NCORES):
        arr = res1.results[c]["h1out"].reshape(128, S, 4, W).astype(np.float32)
        # token (c*W+j)*S + d, hid b*128+p  <- arr[p, d, b, j]
        blk = arr.transpose(3, 1, 2, 0).reshape(W * S, H)     # [(j,d),(b,p)]
        h1_all[c * W * S:(c + 1) * W * S] = blk

    # launch 2
    wfint = np.ascontiguousarray(
        W_fin.T.reshape(36, 128, 512).transpose(1, 0, 2).reshape(128, 36 * 512)
    ).astype(np.float16)
    wly2tab = (W_ly2.T + (b_ly2 / IDX)[None, :]).astype(np.float32)   # [9*E, H]
    bfin = np.zeros((1, 512 + 128), np.float16)
    bfin[0, 0:512] = b_fin.astype(np.float16)
    bfin[0, 512:] = 1.0

    h1_ntok = h1_all.reshape(IDX, B * T, H)
    seq_tok = sequence.reshape(B * T, IDX).astype(np.int64)
    raw_all = wly2tab[(np.arange(IDX)[None, :] * E + seq_tok)].sum(axis=1)  # [B*T, H]

    in_maps2 = []
    for c in range(NCORES):
        sl = slice(c * 128, (c + 1) * 128)
        h1t = h1_ntok[:, sl, :].reshape(IDX, 128, 4, 128) \
            .transpose(3, 0, 2, 1).reshape(128, 36 * 128)
        bfpad = np.zeros((128, 640), np.float16)
        bfpad[0] = bfin[0]
        h1x = np.concatenate([h1t, raw_all[sl].astype(np.float16), bfpad],
                             axis=1).astype(np.float16)
        in_maps2.append({"h1x": np.ascontiguousarray(h1x), "wfint": wfint})

    res2 = _run_with_retry(_cache["l2"], in_maps2)
    out = np.concatenate([res2.results[c]["out"] for c in range(NCORES)], axis=0)
    return np.ascontiguousarray(out.reshape(B, T, H)).astype(np.float32)
